# revision 7
# baseline (speedup 1.0000x reference)
"""Single-head attention (B=4, S=4096, E=1024, H=64) on 8 TRN2 NeuronCores.

Sharding: core c -> (batch b = c//2, sequence half h = c%2). Each core receives
the FULL 4096-row x of its batch, rotated so its own query half comes first
(attention is permutation-invariant over keys, so a per-core key order is
fine). Every core computes K/V for all 4096 rows locally and queries for its
own 2048 rows -- there are NO collectives: the measured CC-stream barrier has
a fixed ~50+us completion floor that an AllGather of K/V cannot beat, while
the redundant K/V projection costs only ~27us of PE time and overlaps the
input stream.

The emission is a software-pipelined schedule: projection chunks (512 rows:
PE transpose -> QKV matmul -> bias) interleave with attention blocks (query
chunk x key group) the moment their inputs exist, and the output projection is
chunked at QC=512 and woven between attention blocks, so the PE stays busy
end-to-end and the HAM clock gate never re-throttles to 1.2 GHz. All x-tile
DMAs are enqueued up-front on both HWDGE queues (16 staging buffers) so input
streams at full HBM rate; weights go first so the projection never waits.

The scalar (ACT) engine runs ONLY the exp activations (it is the secondary
bottleneck); casts/copies/muls run on DVE and GpSimd.

Matmuls run in bf16, accumulating fp32 in PSUM, all with K=128: the kq tile
[K rows 0:64 | Q rows 64:128] is used directly as the scores stationary --
the Q rows meet explicit zeros in the moving operand q2 (memset once), so
they contribute nothing. The softmax denominator rides as row 64 of the
context (ones column in v_aug), is transposed by tiny PE matmuls, and its
reciprocal scales the output projection, whose row 64 of W_out carries b_out
(denom * recip == 1).
"""

import sys

import numpy as np

for _p in ("/opt/trn_rl_repo",):
    if _p not in sys.path:
        sys.path.insert(0, _p)

from contextlib import ExitStack

import concourse.bass as bass  # noqa: F401  (import keeps bass registered)
import concourse.mybir as mybir
import concourse.tile as tile
from concourse import bacc, masks
from concourse.bass_utils import run_bass_kernel_spmd

F32 = mybir.dt.float32
BF16 = mybir.dt.bfloat16
AF = mybir.ActivationFunctionType
ALU = mybir.AluOpType

B, S, E, H = 4, 4096, 1024, 64
D3 = 3 * H            # 192
SH = S // 2           # queries per core
N_CORES = 8
CW = 512              # projection chunk rows
NCH = S // CW         # 8 projection chunks over the full sequence
QC = 512              # query chunk for attention/output projection
NQC = SH // QC        # 4 query chunks
ETILES = E // 128     # 8 embedding tiles
WSTR = 256            # w_sb per-e-tile stride: [K|Q|V|junk] columns
SCALE = 0.125         # 1/sqrt(H)
ST = S // 128         # 32 kj tiles over the full sequence


def _emit(nc, tc, x_ext, wq_ext, bq_ext, wo_ext, bo_ext, out_ext):
    with ExitStack() as top:
        const = top.enter_context(tc.tile_pool(name="const", bufs=1))

        ident = const.tile([128, 128], BF16)
        masks.make_identity(nc, ident[:])

        # ---- weight/bias staging on the HWDGE queues BEFORE the x tiles so
        # the first projection never waits (v2 lesson: SWDGE w-load starved
        # behind x and stalled the pipeline until 43us)
        wstage = top.enter_context(tc.tile_pool(name="wstage", bufs=1))
        w32 = wstage.tile([128, ETILES * D3], F32)
        nc.gpsimd.dma_start(
            w32[:].rearrange("p (e d) -> p e d", d=D3),
            wq_ext.rearrange("(e p) d -> p e d", p=128),
        )
        wo32 = wstage.tile([H, E], F32)
        nc.gpsimd.dma_start(wo32[:], wo_ext[:, :])
        bo32 = wstage.tile([1, E], F32)
        nc.gpsimd.dma_start(bo32[:], bo_ext.unsqueeze(0))
        bkq = const.tile([128, 1], F32)  # [b_k ; b_q]
        nc.gpsimd.dma_start(bkq[0:64, :], bq_ext[64:128].unsqueeze(1))
        nc.gpsimd.dma_start(bkq[64:128, :], bq_ext[0:64].unsqueeze(1))
        bv = const.tile([64, 1], F32)
        nc.gpsimd.dma_start(bv[:], bq_ext[128:192].unsqueeze(1))

        w_sb = const.tile([128, ETILES * WSTR], BF16)
        wo_sb = const.tile([128, E], BF16)
        nc.gpsimd.memset(wo_sb[:], 0.0)

        def emit_wstage():
            # deferred: these vector copies wait on the w32 DMA, so they are
            # emitted after the first x casts to keep the DVE queue flowing
            w_sb_v = w_sb[:].rearrange("p (e c) -> p e c", c=WSTR)
            w32_v = w32[:].rearrange("p (e c) -> p e c", c=D3)
            nc.vector.tensor_copy(w_sb_v[:, :, 0:64], w32_v[:, :, 64:128])
            nc.vector.tensor_copy(w_sb_v[:, :, 64:128], w32_v[:, :, 0:64])
            nc.vector.tensor_copy(w_sb_v[:, :, 128:192], w32_v[:, :, 128:192])
            # W_out: rows 0:64 = W_out, row 64 = b_out, rows 65:128 = 0
            nc.vector.tensor_copy(wo_sb[0:64, :], wo32[:])
            bo16 = wstage.tile([1, E], BF16)
            nc.vector.tensor_copy(bo16[:], bo32[:])
            nc.gpsimd.dma_start(wo_sb[64:65, :], bo16[:])

        # ---- persistent attention operands
        v_aug = const.tile([128, ST * 128], BF16)  # [keys, 64 v | 1 | zeros]
        nc.gpsimd.memset(v_aug[:], 0.0)
        nc.gpsimd.memset(
            v_aug[:].rearrange("p (t c) -> p t c", c=128)[:, :, 64:65], 1.0
        )
        q2_sb = const.tile([128, SH], BF16)  # qT on rows 0:64, zeros below
        nc.gpsimd.memset(q2_sb[:], 0.0)
        ones11 = const.tile([1, 1], BF16)
        nc.gpsimd.memset(ones11[:], 1.0)

        # ---- pools
        xsb = top.enter_context(tc.tile_pool(name="xsb", bufs=6))
        xbp = top.enter_context(tc.tile_pool(name="xbp", bufs=6))
        xTp = top.enter_context(tc.tile_pool(name="xTp", bufs=2))
        kqp = top.enter_context(tc.tile_pool(name="kqp", bufs=NCH))
        vstp = top.enter_context(tc.tile_pool(name="vstp", bufs=2))
        expp = top.enter_context(tc.tile_pool(name="expp", bufs=4))
        cbp = top.enter_context(tc.tile_pool(name="cbp", bufs=2))
        rsp = top.enter_context(tc.tile_pool(name="rsp", bufs=2))
        outp = top.enter_context(tc.tile_pool(name="outp", bufs=3))
        # PSUM banks: sps 2x[128,1024](4) + mqkv [128,1024](2) + cps 2x(2)
        # the sps ring also carries transposes, v/rs transposes and out-proj
        sps = top.enter_context(tc.tile_pool(name="sps", bufs=2, space="PSUM"))
        mqkv = top.enter_context(tc.tile_pool(name="mqkv", bufs=1, space="PSUM"))
        cps = top.enter_context(tc.tile_pool(name="cps", bufs=2, space="PSUM"))

        # ---- x loaded as 8 big 2MB chunk DMAs (the per-DMA completion
        # latency is ~2us, so small transfers serialize the queue); chunk 0
        # leads the scalar queue, w32 leads sync
        t32s = []
        for c in range(NCH):
            t32 = xsb.tile([128, 4 * E], F32, name="t32", tag="t32")
            eng = nc.scalar if c % 2 == 0 else nc.sync
            eng.dma_start(
                t32[:].rearrange("p (t e) -> p t e", e=E),
                x_ext[c * CW:(c + 1) * CW, :].rearrange(
                    "(t p) e -> p t e", p=128
                ),
            )
            t32s.append(t32)

        xbs = {}

        def emit_casts(c):
            for t in range(4):
                k = 4 * c + t
                xb = xbp.tile([128, E], BF16, name="xb", tag="xb")
                nc.vector.tensor_copy(xb[:], t32s[c][:, t * E:(t + 1) * E])
                xbs[k] = xb

        kqs = []
        v_aug_v = v_aug[:].rearrange("p (j c) -> p j c", c=128)

        def emit_tp(c):
            """transpose + project chunk c; fill v_aug, q2 (own chunks)"""
            xT = xTp.tile([128, ETILES * CW], BF16, name="xT", tag="xT")
            for e in range(ETILES):
                p = sps.tile([128, 2 * CW], F32, name="sc", tag="sc")
                for t in range(4):
                    nc.tensor.matmul(
                        p[:, t * 128:(t + 1) * 128],
                        xbs[4 * c + t][:, e * 128:(e + 1) * 128],
                        ident[:],
                    )
                nc.vector.tensor_copy(xT[:, e * CW:(e + 1) * CW], p[:, 0:CW])
            m = mqkv.tile([128, 2 * CW], F32, name="m")
            m1 = m[:, 0:CW]
            m2 = m[:, CW:2 * CW]
            for e in range(ETILES):
                rhs = xT[:, e * CW:(e + 1) * CW]
                nc.tensor.matmul(
                    m1, w_sb[:, e * WSTR:e * WSTR + 128], rhs,
                    start=(e == 0), stop=(e == ETILES - 1),
                    skip_group_check=True,
                )
                nc.tensor.matmul(
                    m2, w_sb[:, e * WSTR + 128:e * WSTR + 256], rhs,
                    start=(e == 0), stop=(e == ETILES - 1),
                    skip_group_check=True,
                )
            # kq = [K rows 0:64 | Q rows 64:128]: used directly as the scores
            # stationary (Q rows hit q2's zero rows 64:128)
            kq = kqp.tile([128, CW], BF16, name="kq", tag="kq")
            nc.vector.tensor_scalar_add(kq[:], m1, bkq[:])
            kqs.append(kq)
            vst = vstp.tile([64, CW], BF16, name="vst", tag="vst")
            nc.vector.tensor_scalar_add(vst[:], m2[0:64, :], bv[:])
            # v to natural [keys, h] layout straight into v_aug
            vp = sps.tile([128, 2 * CW], F32, name="sc", tag="sc")
            for t in range(4):
                nc.tensor.matmul(
                    vp[:, t * 64:(t + 1) * 64],
                    vst[:, t * 128:(t + 1) * 128],
                    ident[0:64, 0:64],
                )
            nc.vector.tensor_copy(
                v_aug_v[:, c * 4:c * 4 + 4, 0:64],
                vp[:, 0:256].rearrange("p (t c) -> p t c", c=64),
            )
            if c < NQC:  # own-query chunk: extract q columns
                nc.sync.dma_start(
                    q2_sb[0:64, c * CW:(c + 1) * CW], kq[64:128, :]
                )

        ctxs = {}
        kj_count = {}

        def emit_attn(xa, g):
            """attention block: query chunks (xa, xa+1) against key group g;
            one exp instruction covers both chunks (ACT is the tight engine)"""
            for x in (xa, xa + 1):
                if x not in ctxs:
                    ctxs[x] = cps.tile([128, QC], F32, name="ctx", tag="ctx")
                    kj_count[x] = 0
            q0 = xa * QC
            kq = kqs[g]
            for t in range(4):
                j = g * 4 + t
                sc = sps.tile([128, 2 * QC], F32, name="sc", tag="sc")
                nc.tensor.matmul(
                    sc[:, 0:QC],
                    kq[:, t * 128:(t + 1) * 128],
                    q2_sb[:, q0:q0 + QC],
                )
                nc.tensor.matmul(
                    sc[:, QC:2 * QC],
                    kq[:, t * 128:(t + 1) * 128],
                    q2_sb[:, q0 + QC:q0 + 2 * QC],
                )
                ex = expp.tile([128, 2 * QC], BF16, name="ex", tag="ex")
                nc.scalar.activation(ex[:], sc[:], AF.Exp, scale=SCALE)
                for x in (xa, xa + 1):
                    i = kj_count[x]
                    kj_count[x] += 1
                    nc.tensor.matmul(
                        ctxs[x][:],
                        v_aug[:, j * 128:(j + 1) * 128],
                        ex[:, (x - xa) * QC:(x - xa + 1) * QC],
                        start=(i == 0), stop=(i == ST - 1),
                        skip_group_check=True,
                    )

        out_q = [nc.sync, nc.scalar]

        def emit_phc(x):
            """output projection for query chunk x"""
            ctx = ctxs.pop(x)
            q0 = x * QC
            # rows 65:128 of ctx are exact zeros (v_aug zero padding), so the
            # bf16 copy takes all 128 rows and the out matmul K=128 is safe
            ctx16 = cbp.tile([128, QC], BF16, name="ctx16", tag="ctx16")
            nc.vector.tensor_copy(ctx16[:], ctx[:])
            rs_row = rsp.tile([1, QC], BF16, name="rs_row", tag="rs_row")
            nc.vector.tensor_copy(rs_row[:], ctx16[64:65, :])
            rs_ps = sps.tile([128, 2 * CW], F32, name="sc", tag="sc")
            for i in range(QC // 128):
                nc.tensor.matmul(
                    rs_ps[:, i:i + 1],
                    rs_row[0:1, i * 128:(i + 1) * 128],
                    ones11[:],
                )
            recip = rsp.tile([128, QC // 128], F32, name="recip", tag="recip")
            nc.vector.reciprocal(recip[:], rs_ps[:, 0:QC // 128])
            for i in range(QC // 128):
                out_sb = outp.tile([128, E], F32, name="out_sb", tag="out_sb")
                for n in range(2):
                    op = sps.tile([128, 2 * CW], F32, name="sc", tag="sc")
                    nc.tensor.matmul(
                        op[:, 0:CW],
                        ctx16[:, i * 128:(i + 1) * 128],
                        wo_sb[:, n * 512:(n + 1) * 512],
                    )
                    nc.vector.tensor_scalar_mul(
                        out_sb[:, n * 512:(n + 1) * 512], op[:, 0:CW],
                        recip[:, i:i + 1],
                    )
                out_q[i % 2].dma_start(
                    out_ext[q0 + i * 128:q0 + (i + 1) * 128, :], out_sb[:]
                )

        # ---- the schedule: projection chunks with paired attention blocks
        # woven in as soon as their inputs exist
        emit_casts(0)
        emit_casts(1)
        emit_wstage()
        emit_tp(0)
        emit_casts(2)
        emit_tp(1)
        emit_attn(0, 0)
        emit_casts(3)
        emit_tp(2)
        emit_attn(0, 1)
        emit_casts(4)
        emit_tp(3)
        emit_attn(0, 2)
        emit_casts(5)
        emit_tp(4)
        emit_attn(0, 3)
        emit_casts(6)
        emit_tp(5)
        emit_attn(0, 4)
        emit_casts(7)
        emit_tp(6)
        emit_attn(0, 5)
        emit_attn(0, 6)
        emit_tp(7)
        emit_attn(0, 7)
        emit_phc(0)
        emit_phc(1)
        for g in range(NCH):
            emit_attn(2, g)
        emit_phc(2)
        emit_phc(3)

_NC = None


def _get_nc():
    global _NC
    if _NC is None:
        nc = bacc.Bacc("TRN2", target_bir_lowering=False, debug=False,
                       num_devices=N_CORES)
        x_ext = nc.dram_tensor("x", [S, E], F32, kind="ExternalInput").ap()
        wq_ext = nc.dram_tensor("w_qkv", [E, D3], F32, kind="ExternalInput").ap()
        bq_ext = nc.dram_tensor("b_qkv", [D3], F32, kind="ExternalInput").ap()
        wo_ext = nc.dram_tensor("w_out", [H, E], F32, kind="ExternalInput").ap()
        bo_ext = nc.dram_tensor("b_out", [E], F32, kind="ExternalInput").ap()
        out_ext = nc.dram_tensor("out", [SH, E], F32, kind="ExternalOutput").ap()
        with tile.TileContext(nc) as tc:
            _emit(nc, tc, x_ext, wq_ext, bq_ext, wo_ext, bo_ext, out_ext)
        nc.compile()
        _NC = nc
    return _NC


last_results = None
last_tmpdir = None


def kernel(x, W_qkv, b_qkv, W_out, b_out):
    nc = _get_nc()
    x = np.ascontiguousarray(x, dtype=np.float32)
    shared = {
        "w_qkv": np.ascontiguousarray(W_qkv, dtype=np.float32),
        "b_qkv": np.ascontiguousarray(b_qkv, dtype=np.float32),
        "w_out": np.ascontiguousarray(W_out, dtype=np.float32),
        "b_out": np.ascontiguousarray(b_out, dtype=np.float32),
    }
    in_maps = []
    for c in range(N_CORES):
        b, h = divmod(c, 2)
        # full batch row, rotated so the core's own query half comes first
        # (key order is a per-core permutation; softmax doesn't care)
        xp = np.ascontiguousarray(
            np.concatenate(
                [x[b, h * SH:(h + 1) * SH], x[b, (1 - h) * SH:(2 - h) * SH]]
            )
        )
        in_maps.append({"x": xp, **shared})

    import os
    import tempfile
    import time

    tmpdir = os.environ.get("ATTN_TRACE_DIR") or tempfile.mkdtemp(prefix="attn_trace_")
    res = None
    for attempt in range(3):
        try:
            res = run_bass_kernel_spmd(
                nc, in_maps, core_ids=list(range(N_CORES)), tmpdir=tmpdir
            )
            break
        except Exception:
            # transient NRT_EXEC_UNIT_UNRECOVERABLE has been observed on a
            # first attempt; a clean retry recovers
            if attempt == 2:
                raise
            time.sleep(2.0)
    global last_results, last_tmpdir
    last_results = res
    last_tmpdir = tmpdir

    out = np.empty((B, S, E), dtype=np.float32)
    for c in range(N_CORES):
        b, h = divmod(c, 2)
        out[b, h * SH:(h + 1) * SH] = res.results[c]["out"]
    return out


# revision 9
# speedup vs baseline: 1.3586x; 1.3586x over previous
"""Single-head attention (B=4, S=4096, E=1024, H=64) on 8 TRN2 NeuronCores.

Sharding: core c -> (batch b = c//2, sequence half h = c%2). Each core receives
the FULL 4096-row x of its batch, rotated so its own query half comes first
(attention is permutation-invariant over keys, so a per-core key order is
fine). Every core computes K/V for all 4096 rows locally and queries for its
own 2048 rows -- there are NO collectives: the measured CC-stream barrier has
a fixed ~50+us completion floor that an AllGather of K/V cannot beat, while
the redundant K/V projection overlaps the input stream.

Measured engine facts this schedule is built around:
- exp on the ACT engine is ~650ns per [128,512] tile (~9ps/elem): the 8.4M
  softmax elements cost ~83us, MORE than the 54.6us of attention PE work, so
  attention alone is ACT-bound. The schedule therefore round-robins attention
  blocks of all four query chunks and weaves the projection chunks and output
  projections between them, keeping the PE (the true global bottleneck at
  ~120us of work) busy through the ACT gaps.
- context is accumulated per 4-tile block in PSUM and added into an SBUF
  fp32 accumulator (DVE), so a PSUM bank is not pinned per query chunk and
  any block order is legal.
- per-DMA completion latency is ~2us and sub-KB descriptors starve the SDMA
  engines, so x loads as 8 2MB chunk DMAs, weights as one 3D DMA on the idle
  SWDGE ring, and the qkv bias as a single 192-element row that two tiny PE
  matmuls transpose onto partitions.
- engine queues are strict FIFO (a waiting instruction blocks everything
  behind it), so the emission order below is the execution order per engine.

Matmuls run in bf16, accumulating fp32 in PSUM, all with K=128: the kq tile
[Q rows 0:64 | K rows 64:128] is used directly as the scores stationary --
the Q rows meet explicit zeros in the moving operand q2 (memset once), so
they contribute nothing. The softmax denominator rides as row 64 of the
context (ones column in v_aug), is transposed by tiny PE matmuls, and its
reciprocal scales the output projection, whose row 64 of W_out carries b_out
(denom * recip == 1).
"""

import sys

import numpy as np

for _p in ("/opt/trn_rl_repo",):
    if _p not in sys.path:
        sys.path.insert(0, _p)

from contextlib import ExitStack

import concourse.bass as bass  # noqa: F401  (import keeps bass registered)
import concourse.mybir as mybir
import concourse.tile as tile
from concourse import bacc, masks
from concourse.bass_utils import run_bass_kernel_spmd

F32 = mybir.dt.float32
BF16 = mybir.dt.bfloat16
AF = mybir.ActivationFunctionType
ALU = mybir.AluOpType

B, S, E, H = 4, 4096, 1024, 64
D3 = 3 * H            # 192
SH = S // 2           # queries per core
N_CORES = 8
CW = 512              # projection chunk rows
NCH = S // CW         # 8 projection chunks over the full sequence
QC = 512              # query chunk for attention/output projection
NQC = SH // QC        # 4 query chunks
ETILES = E // 128     # 8 embedding tiles
WSTR = 256            # w_sb per-e-tile stride: [Q|K|V|junk] columns
SCALE = 0.125         # 1/sqrt(H)
ST = S // 128         # 32 kj tiles over the full sequence


def _emit(nc, tc, x_ext, wq_ext, bq_ext, wo_ext, bo_ext, out_ext):
    with ExitStack() as top:
        const = top.enter_context(tc.tile_pool(name="const", bufs=1))

        ident = const.tile([128, 128], BF16)
        masks.make_identity(nc, ident[:])

        # ---- staging DMAs: w32 leads the SWDGE ring, bias as one 192-elem
        # row (per-partition 4B-descriptor DMAs starve the SDMA engines)
        wstage = top.enter_context(tc.tile_pool(name="wstage", bufs=1))
        w32 = wstage.tile([128, ETILES * D3], F32)
        nc.gpsimd.dma_start(
            w32[:].rearrange("p (e d) -> p e d", d=D3),
            wq_ext.rearrange("(e p) d -> p e d", p=128),
        )
        bq_row = wstage.tile([1, D3], F32)
        nc.gpsimd.dma_start(bq_row[:], bq_ext.unsqueeze(0))
        q2_sb = const.tile([128, SH], BF16)  # qT on rows 64:128, zeros above
        nc.gpsimd.memset(q2_sb[:], 0.0)
        v_aug = const.tile([128, ST * 128], BF16)  # [keys, 64 v | 1 | zeros]
        nc.gpsimd.memset(v_aug[:], 0.0)
        nc.gpsimd.memset(
            v_aug[:].rearrange("p (t c) -> p t c", c=128)[:, :, 64:65], 1.0
        )
        ones11 = const.tile([1, 1], BF16)
        nc.gpsimd.memset(ones11[:], 1.0)
        ones11f = const.tile([1, 1], F32)
        nc.gpsimd.memset(ones11f[:], 1.0)
        wo_sb = const.tile([128, E], BF16)
        nc.gpsimd.memset(wo_sb[:], 0.0)
        wo32 = wstage.tile([H, E], F32)
        nc.gpsimd.dma_start(wo32[:], wo_ext[:, :])
        bo32 = wstage.tile([1, E], F32)
        nc.gpsimd.dma_start(bo32[:], bo_ext.unsqueeze(0))

        w_sb = const.tile([128, ETILES * WSTR], BF16)
        bkq = const.tile([128, 1], F32)   # [b_q ; b_k] on partitions
        bv = const.tile([64, 1], F32)

        # ---- pools
        xsb = top.enter_context(tc.tile_pool(name="xsb", bufs=6))
        xbp = top.enter_context(tc.tile_pool(name="xbp", bufs=6))
        xTp = top.enter_context(tc.tile_pool(name="xTp", bufs=2))
        kqp = top.enter_context(tc.tile_pool(name="kqp", bufs=NCH))
        vstp = top.enter_context(tc.tile_pool(name="vstp", bufs=2))
        accp = top.enter_context(tc.tile_pool(name="accp", bufs=4))
        expp = top.enter_context(tc.tile_pool(name="expp", bufs=6))
        cbp = top.enter_context(tc.tile_pool(name="cbp", bufs=2))
        rsp = top.enter_context(tc.tile_pool(name="rsp", bufs=2))
        outp = top.enter_context(tc.tile_pool(name="outp", bufs=3))
        # PSUM banks: xtp(2) + m1p(1) + m2p(1) + sps(2) + cps(2) = 8
        xtp = top.enter_context(tc.tile_pool(name="xtp", bufs=2, space="PSUM"))
        m1p = top.enter_context(tc.tile_pool(name="m1p", bufs=1, space="PSUM"))
        m2p = top.enter_context(tc.tile_pool(name="m2p", bufs=1, space="PSUM"))
        sps = top.enter_context(tc.tile_pool(name="sps", bufs=2, space="PSUM"))
        cps = top.enter_context(tc.tile_pool(name="cps", bufs=2, space="PSUM"))

        # ---- all x chunk DMAs up-front on both HWDGE queues
        t32s = []
        for c in range(NCH):
            t32 = xsb.tile([128, 4 * E], F32, name="t32", tag="t32")
            eng = nc.scalar if c % 2 == 0 else nc.sync
            eng.dma_start(
                t32[:].rearrange("p (t e) -> p t e", e=E),
                x_ext[c * CW:(c + 1) * CW, :].rearrange(
                    "(t p) e -> p t e", p=128
                ),
            )
            t32s.append(t32)

        xbs = {}

        def emit_casts(c):
            for t in range(4):
                xb = xbp.tile([128, E], BF16, name="xb", tag="xb")
                nc.vector.tensor_copy(xb[:], t32s[c][:, t * E:(t + 1) * E])
                xbs[4 * c + t] = xb

        def emit_wstage():
            # deferred past the first x casts: vector is strict FIFO and
            # these wait on the w32/bias DMAs
            w_sb_v = w_sb[:].rearrange("p (e c) -> p e c", c=WSTR)
            w32_v = w32[:].rearrange("p (e c) -> p e c", c=D3)
            nc.vector.tensor_copy(w_sb_v[:, :, 0:D3], w32_v[:])  # [Q|K|V]
            # bias onto partitions via two tiny PE transposes
            bp = xtp.tile([128, CW], F32, name="xtps", tag="xtps")
            nc.tensor.matmul(bp[:, 0:1], bq_row[0:1, 0:128], ones11f[:])
            nc.tensor.matmul(bp[0:64, 1:2], bq_row[0:1, 128:192], ones11f[:])
            nc.vector.tensor_copy(bkq[:], bp[:, 0:1])
            nc.vector.tensor_copy(bv[:], bp[0:64, 1:2])
            nc.vector.tensor_copy(wo_sb[0:64, :], wo32[:])
            nc.vector.tensor_copy(wo_sb[64:65, :], bo32[:])

        kqs = []
        v_aug_v = v_aug[:].rearrange("p (j c) -> p j c", c=128)

        def emit_tp(c):
            """transpose + project chunk c; fill v_aug, q2 (own chunks)"""
            xT = xTp.tile([128, ETILES * CW], BF16, name="xT", tag="xT")
            for e in range(ETILES):
                p = xtp.tile([128, CW], F32, name="xtps", tag="xtps")
                for t in range(4):
                    nc.tensor.matmul(
                        p[:, t * 128:(t + 1) * 128],
                        xbs[4 * c + t][:, e * 128:(e + 1) * 128],
                        ident[:],
                    )
                nc.vector.tensor_copy(xT[:, e * CW:(e + 1) * CW], p[:])
            m1 = m1p.tile([128, CW], F32, name="m1")
            m2 = m2p.tile([128, CW], F32, name="m2")
            for e in range(ETILES):
                rhs = xT[:, e * CW:(e + 1) * CW]
                nc.tensor.matmul(
                    m1[:], w_sb[:, e * WSTR:e * WSTR + 128], rhs,
                    start=(e == 0), stop=(e == ETILES - 1),
                )
                nc.tensor.matmul(
                    m2[:], w_sb[:, e * WSTR + 128:e * WSTR + 256], rhs,
                    start=(e == 0), stop=(e == ETILES - 1),
                )
            # kq = [Q rows 0:64 | K rows 64:128]: used directly as the scores
            # stationary (Q rows hit q2's zero rows 0:64)
            kq = kqp.tile([128, CW], BF16, name="kq", tag="kq")
            nc.vector.tensor_scalar_add(kq[:], m1[:], bkq[:])
            kqs.append(kq)
            vst = vstp.tile([64, CW], BF16, name="vst", tag="vst")
            nc.vector.tensor_scalar_add(vst[:], m2[0:64, :], bv[:])
            # v to natural [keys, h] layout straight into v_aug
            vp = xtp.tile([128, CW], F32, name="xtps", tag="xtps")
            for t in range(4):
                nc.tensor.matmul(
                    vp[:, t * 64:(t + 1) * 64],
                    vst[:, t * 128:(t + 1) * 128],
                    ident[0:64, 0:64],
                )
            nc.vector.tensor_copy(
                v_aug_v[:, c * 4:c * 4 + 4, 0:64],
                vp[:, 0:256].rearrange("p (t c) -> p t c", c=64),
            )
            if c < NQC:  # own-query chunk: extract q columns
                nc.sync.dma_start(
                    q2_sb[64:128, c * CW:(c + 1) * CW], kq[0:64, :]
                )

        accs = {}

        def emit_attn(x, g):
            """attention block: query chunk x against key group g (4 kj);
            block context accumulates in PSUM, then adds into an SBUF
            accumulator so no PSUM bank is pinned per query chunk"""
            q0 = x * QC
            kq = kqs[g]
            bctx = cps.tile([128, QC], F32, name="bctx", tag="bctx")
            for t in range(4):
                j = g * 4 + t
                sc = sps.tile([128, QC], F32, name="sc", tag="sc")
                nc.tensor.matmul(
                    sc[:],
                    kq[:, t * 128:(t + 1) * 128],
                    q2_sb[:, q0:q0 + QC],
                )
                ex = expp.tile([128, QC], BF16, name="ex", tag="ex")
                nc.scalar.activation(ex[:], sc[:], AF.Exp, scale=SCALE)
                nc.tensor.matmul(
                    bctx[:],
                    v_aug[:, j * 128:(j + 1) * 128],
                    ex[:],
                    start=(t == 0), stop=(t == 3),
                )
            if x not in accs:
                accs[x] = accp.tile([128, QC], F32, name="acc", tag="acc")
                nc.vector.tensor_copy(accs[x][:], bctx[:])
            else:
                nc.vector.tensor_add(accs[x][:], accs[x][:], bctx[:])

        out_q = [nc.sync, nc.scalar]

        def emit_phc(x):
            """output projection for query chunk x"""
            acc = accs.pop(x)
            q0 = x * QC
            # rows 65:128 of acc are exact zeros (v_aug zero padding), so the
            # bf16 copy takes all 128 rows and the out matmul K=128 is safe
            ctx16 = cbp.tile([128, QC], BF16, name="ctx16", tag="ctx16")
            nc.vector.tensor_copy(ctx16[:], acc[:])
            rs_row = rsp.tile([1, QC], BF16, name="rs_row", tag="rs_row")
            nc.vector.tensor_copy(rs_row[:], ctx16[64:65, :])
            rs_ps = xtp.tile([128, CW], F32, name="xtps", tag="xtps")
            for i in range(QC // 128):
                nc.tensor.matmul(
                    rs_ps[:, i:i + 1],
                    rs_row[0:1, i * 128:(i + 1) * 128],
                    ones11[:],
                )
            recip = rsp.tile([128, QC // 128], F32, name="recip", tag="recip")
            nc.vector.reciprocal(recip[:], rs_ps[:, 0:QC // 128])
            for i in range(QC // 128):
                out_sb = outp.tile([128, E], F32, name="out_sb", tag="out_sb")
                for n in range(2):
                    op = xtp.tile([128, CW], F32, name="xtps", tag="xtps")
                    nc.tensor.matmul(
                        op[:],
                        ctx16[:, i * 128:(i + 1) * 128],
                        wo_sb[:, n * 512:(n + 1) * 512],
                    )
                    nc.vector.tensor_scalar_mul(
                        out_sb[:, n * 512:(n + 1) * 512], op[:],
                        recip[:, i:i + 1],
                    )
                out_q[i % 2].dma_start(
                    out_ext[q0 + i * 128:q0 + (i + 1) * 128, :], out_sb[:]
                )

        # ---- the schedule: projection chunks early (paced by the x stream),
        # attention blocks round-robined across query chunks, output
        # projections woven into the ACT-bound late region
        emit_casts(0)
        emit_casts(1)
        emit_wstage()
        emit_tp(0)
        emit_casts(2)
        emit_tp(1)
        emit_attn(0, 0)
        emit_casts(3)
        emit_tp(2)
        emit_attn(1, 0)
        emit_attn(0, 1)
        emit_casts(4)
        emit_tp(3)
        emit_attn(2, 0)
        emit_attn(1, 1)
        emit_casts(5)
        emit_tp(4)
        emit_attn(3, 0)
        emit_attn(0, 2)
        emit_casts(6)
        emit_tp(5)
        emit_attn(2, 1)
        emit_attn(1, 2)
        emit_casts(7)
        emit_tp(6)
        emit_attn(3, 1)
        emit_attn(0, 3)
        emit_tp(7)
        for x, g in [(2, 2), (1, 3), (0, 4), (3, 2), (2, 3), (1, 4), (0, 5),
                     (3, 3), (2, 4), (1, 5), (0, 6), (3, 4), (2, 5), (1, 6),
                     (0, 7)]:
            emit_attn(x, g)
        emit_phc(0)
        for x, g in [(3, 5), (2, 6), (1, 7)]:
            emit_attn(x, g)
        emit_phc(1)
        emit_attn(3, 6)
        emit_attn(2, 7)
        emit_phc(2)
        emit_attn(3, 7)
        emit_phc(3)


_NC = None


def _get_nc():
    global _NC
    if _NC is None:
        nc = bacc.Bacc("TRN2", target_bir_lowering=False, debug=False,
                       num_devices=N_CORES)
        x_ext = nc.dram_tensor("x", [S, E], F32, kind="ExternalInput").ap()
        wq_ext = nc.dram_tensor("w_qkv", [E, D3], F32, kind="ExternalInput").ap()
        bq_ext = nc.dram_tensor("b_qkv", [D3], F32, kind="ExternalInput").ap()
        wo_ext = nc.dram_tensor("w_out", [H, E], F32, kind="ExternalInput").ap()
        bo_ext = nc.dram_tensor("b_out", [E], F32, kind="ExternalInput").ap()
        out_ext = nc.dram_tensor("out", [SH, E], F32, kind="ExternalOutput").ap()
        with tile.TileContext(nc) as tc:
            _emit(nc, tc, x_ext, wq_ext, bq_ext, wo_ext, bo_ext, out_ext)
        nc.compile()
        _NC = nc
    return _NC


last_results = None
last_tmpdir = None


def kernel(x, W_qkv, b_qkv, W_out, b_out):
    nc = _get_nc()
    x = np.ascontiguousarray(x, dtype=np.float32)
    shared = {
        "w_qkv": np.ascontiguousarray(W_qkv, dtype=np.float32),
        "b_qkv": np.ascontiguousarray(b_qkv, dtype=np.float32),
        "w_out": np.ascontiguousarray(W_out, dtype=np.float32),
        "b_out": np.ascontiguousarray(b_out, dtype=np.float32),
    }
    in_maps = []
    for c in range(N_CORES):
        b, h = divmod(c, 2)
        # full batch row, rotated so the core's own query half comes first
        # (key order is a per-core permutation; softmax doesn't care)
        xp = np.ascontiguousarray(
            np.concatenate(
                [x[b, h * SH:(h + 1) * SH], x[b, (1 - h) * SH:(2 - h) * SH]]
            )
        )
        in_maps.append({"x": xp, **shared})

    import os
    import tempfile
    import time

    tmpdir = os.environ.get("ATTN_TRACE_DIR") or tempfile.mkdtemp(prefix="attn_trace_")
    res = None
    for attempt in range(3):
        try:
            res = run_bass_kernel_spmd(
                nc, in_maps, core_ids=list(range(N_CORES)), tmpdir=tmpdir
            )
            break
        except Exception:
            # transient NRT_EXEC_UNIT_UNRECOVERABLE has been observed on a
            # first attempt; a clean retry recovers
            if attempt == 2:
                raise
            time.sleep(2.0)
    global last_results, last_tmpdir
    last_results = res
    last_tmpdir = tmpdir

    out = np.empty((B, S, E), dtype=np.float32)
    for c in range(N_CORES):
        b, h = divmod(c, 2)
        out[b, h * SH:(h + 1) * SH] = res.results[c]["out"]
    return out


# revision 11
# speedup vs baseline: 1.5138x; 1.1142x over previous
"""Single-head attention (B=4, S=4096, E=1024, H=64) on 8 TRN2 NeuronCores.

Sharding: core c -> (batch b = c//2, sequence half h = c%2). Each core receives
only its own 2048-row x half, computes Q/K/V for it, and the core pair
(2b, 2b+1) exchanges K/V halves with a 2-rank AllGather (two chunked AGs,
overlapped with the projection and the first attention tiles). Every core then
holds K/V for the full 4096-row sequence in global order and computes
attention for its 2048 queries.

Matmuls run in bf16 (fp32 lowers to two LOW_HIGH PE passes on TRN2 — half
throughput); accumulation is fp32 in PSUM, the softmax denominator and the
normalization stay fp32. All matmuls are zero-padded to full 128x128
stationary tiles: masked sub-tile matmuls (K=64 / M=65) leave the PE
clock-gated at 1.2 GHz (HAM does not see them as activity), while full tiles
keep it at 2.4 GHz; the padding costs no extra stream cycles.

Output projection: W_out is padded with b_out as row 64 and the bf16 context
carries the softmax denominator in row 64, so (ctx_aug.T @ W_out_aug) *
recip(denom) applies scale and bias in one pass (denom * recip == 1).

Changes over the original two-phase version, from trace analysis:
- the phase-B constants (kt zero rows, v_aug ones, W_out staging) are emitted
  BEFORE the AllGather readouts on the gpsimd queue: they used to sit behind
  the readout that waits for the last AG (~80us), gating the first scores
  matmul at ~83us; now attention starts as soon as AG0's readout lands.
- the second query chunk runs attention+output-projection in two 512-wide
  subchunks: the exposed serial tail after the last context matmul (which ran
  at 1.2 GHz because the HAM clock gate re-throttles during the ~4us scalar
  chain) shrinks by half.
- the scalar (ACT) engine does only exp in the attention region; casts,
  PSUM copies, and output scaling run on DVE so exp is never queued behind
  them (exp is the attention-phase rate limiter at ~1.15us per [128,1024]).
"""

import sys

import numpy as np

for _p in ("/opt/trn_rl_repo",):
    if _p not in sys.path:
        sys.path.insert(0, _p)

from contextlib import ExitStack

import concourse.bass as bass  # noqa: F401  (import keeps bass registered)
import concourse.mybir as mybir
import concourse.tile as tile
from concourse import bacc, masks
from concourse.bass_utils import run_bass_kernel_spmd

F32 = mybir.dt.float32
BF16 = mybir.dt.bfloat16
AF = mybir.ActivationFunctionType
ALU = mybir.AluOpType

B, S, E, H = 4, 4096, 1024, 64
D3 = 3 * H            # 192
SH = S // 2           # queries per core
N_CORES = 8
QC = 1024             # first query chunk (PSUM-sized)
ST = S // 128         # 32 kj tiles over the full sequence
ETILES = E // 128     # 8 embedding tiles
WSTR = 256            # w_sb per-e-tile stride: [K|Q|V|0] columns
CW = 512              # phase-A chunk width (columns of the own half)
NCH = SH // CW        # 4 chunks, one AllGather each
SCALE = 0.125         # 1/sqrt(H)
WKV = 64 * CW * 2     # AG payload elems per chunk: kT[64,CW] + vT[64,CW]
REPLICA_GROUPS = [[0, 1], [2, 3], [4, 5], [6, 7]]


def _emit(nc, tc, x_ext, wq_ext, bq_ext, wo_ext, bo_ext, out_ext):
    with ExitStack() as top:
        const = top.enter_context(tc.tile_pool(name="const", bufs=1))

        # Critical path first: identity (needed by the first transposes) and
        # the QKV weight staging.
        ident = const.tile([128, 128], BF16)
        masks.make_identity(nc, ident[:])

        # Weights: DMA fp32 staging -> cast to bf16.
        wstage_ctx = ExitStack()
        wstage = wstage_ctx.enter_context(tc.tile_pool(name="wstage", bufs=1))
        w32 = wstage.tile([128, ETILES * D3], F32)
        nc.gpsimd.dma_start(
            w32[:].rearrange("p (e d) -> p e d", d=D3),
            wq_ext.rearrange("(e p) d -> p e d", p=128),
        )
        w_sb = const.tile([128, ETILES * WSTR], BF16)
        w_sb_v = w_sb[:].rearrange("p (e c) -> p e c", c=WSTR)
        w32_v = w32[:].rearrange("p (e c) -> p e c", c=D3)
        nc.vector.tensor_copy(w_sb_v[:, :, 0:64], w32_v[:, :, 64:128])     # K
        nc.vector.tensor_copy(w_sb_v[:, :, 64:128], w32_v[:, :, 0:64])     # Q
        nc.vector.tensor_copy(w_sb_v[:, :, 128:192], w32_v[:, :, 128:192])  # V

        bkq = const.tile([128, 1], F32)  # [b_k ; b_q]
        nc.gpsimd.dma_start(bkq[0:64, :], bq_ext[64:128].unsqueeze(1))
        nc.gpsimd.dma_start(bkq[64:128, :], bq_ext[0:64].unsqueeze(1))
        bv = const.tile([64, 1], F32)
        nc.gpsimd.dma_start(bv[:], bq_ext[128:192].unsqueeze(1))

        # Persistent bf16 matmul operands (global kv order on the free axis)
        kt_sb = const.tile([128, S], BF16)   # kT on 0:64, zeros on 64:128
        vT_sb = const.tile([64, S], BF16)
        q2_sb = const.tile([128, SH], BF16)  # qT on 0:64, zeros on 64:128
        nc.gpsimd.memset(q2_sb[:], 0.0)
        v_aug = const.tile([128, ST * 128], BF16)
        ones11 = const.tile([1, 1], BF16)
        wo_sb = const.tile([128, E], BF16)

        # Phase-B constants EARLY (they used to sit behind the AG readouts on
        # the gpsimd queue and gated the first scores matmul by ~12us)
        nc.gpsimd.memset(kt_sb[64:128, :], 0.0)
        nc.gpsimd.memset(ones11[:], 1.0)
        nc.gpsimd.memset(
            v_aug[:].rearrange("p (t c) -> p t c", c=128)[:, :, 64:65], 1.0
        )
        wo32 = wstage.tile([H, E], F32)
        nc.gpsimd.dma_start(wo32[:], wo_ext[:, :])
        bo32 = wstage.tile([1, E], F32)
        nc.gpsimd.dma_start(bo32[:], bo_ext.unsqueeze(0))
        bo16 = wstage.tile([1, E], BF16)
        nc.vector.tensor_copy(bo16[:], bo32[:])
        nc.gpsimd.memset(wo_sb[:], 0.0)
        nc.vector.tensor_copy(wo_sb[0:64, :], wo32[:])
        nc.gpsimd.dma_start(wo_sb[64:65, :], bo16[:])

        # Small PSUM pool shared by phase-A v-transposes and phase-C tiles
        mps = top.enter_context(tc.tile_pool(name="mps", bufs=2, space="PSUM"))

        # Collective bounce buffers (per AG chunk)
        dram = top.enter_context(tc.tile_pool(name="ccdram", bufs=1, space="DRAM"))
        cc_in = [dram.tile([1, WKV], BF16, name=f"cc_in{c}") for c in range(NCH)]
        cc_out = [dram.tile([2, WKV], BF16, name=f"cc_out{c}") for c in range(NCH)]

        # ---- Phase A: per own-half s-chunk: cast, PE-transpose, project,
        # stage K/V into the pair AllGather --------------------------------
        with ExitStack() as pa:
            xsb = pa.enter_context(tc.tile_pool(name="xsb", bufs=12))
            xbp = pa.enter_context(tc.tile_pool(name="xbp", bufs=8))
            xTp = pa.enter_context(tc.tile_pool(name="xTp", bufs=3))
            stg = pa.enter_context(tc.tile_pool(name="stg", bufs=5))
            kqs = []
            xtp = pa.enter_context(tc.tile_pool(name="xtp", bufs=2, space="PSUM"))
            m1p = pa.enter_context(tc.tile_pool(name="m1p", bufs=2, space="PSUM"))
            m2p = pa.enter_context(tc.tile_pool(name="m2p", bufs=2, space="PSUM"))

            for sc in range(NCH):              # own-half s chunks of CW cols
                xbs = []
                for si in range(CW // 128):
                    st = sc * (CW // 128) + si
                    t32 = xsb.tile([128, E], F32)
                    # split each tile across both HWDGE queues and cast each
                    # half as soon as it lands
                    nc.sync.dma_start(
                        t32[0:64, :], x_ext[st * 128 : st * 128 + 64, :]
                    )
                    nc.scalar.dma_start(
                        t32[64:128, :], x_ext[st * 128 + 64 : (st + 1) * 128, :]
                    )
                    tb = xbp.tile([128, E], BF16)
                    nc.vector.tensor_copy(tb[:], t32[:])
                    xbs.append(tb)
                xT_sc = xTp.tile([128, ETILES * CW], BF16)
                for e in range(ETILES):
                    p = xtp.tile([128, CW], F32)
                    for si in range(CW // 128):
                        nc.tensor.matmul(
                            p[:, si * 128 : (si + 1) * 128],
                            xbs[si][:, e * 128 : (e + 1) * 128],
                            ident[:],
                        )
                    nc.vector.tensor_copy(xT_sc[:, e * CW : (e + 1) * CW], p[:])

                m1 = m1p.tile([128, CW], F32)
                m2 = m2p.tile([128, CW], F32)
                for e in range(ETILES):
                    lhs1 = w_sb[:, e * WSTR : e * WSTR + 128]
                    lhs2 = w_sb[:, e * WSTR + 128 : e * WSTR + 256]
                    rhs = xT_sc[:, e * CW : (e + 1) * CW]
                    nc.tensor.matmul(
                        m1[:], lhs1, rhs,
                        start=(e == 0), stop=(e == ETILES - 1),
                    )
                    nc.tensor.matmul(
                        m2[:], lhs2, rhs,
                        start=(e == 0), stop=(e == ETILES - 1),
                    )
                kq = stg.tile([128, CW], BF16, tag="kq")
                kqs.append(kq)
                nc.vector.tensor_scalar_add(kq[:], m1[:], bkq[:])
                vst = stg.tile([64, CW], BF16, tag="vst")
                nc.vector.tensor_scalar_add(vst[:], m2[0:64, :], bv[:])

                # stage into the AG (gpsimd stream only)
                nc.gpsimd.dma_start(cc_in[sc][0, 0 : 64 * CW], kq[0:64, :])
                nc.gpsimd.dma_start(cc_in[sc][0, 64 * CW : WKV], vst[:])
                nc.gpsimd.collective_compute(
                    "AllGather",
                    ALU.bypass,
                    replica_groups=REPLICA_GROUPS,
                    ins=[cc_in[sc].opt()],
                    outs=[cc_out[sc].opt()],
                )

            # q2 moves and AG readouts AFTER every CC issue, all on the
            # gpsimd stream
            for sc in range(NCH):
                nc.gpsimd.dma_start(
                    q2_sb[0:64, sc * CW : (sc + 1) * CW], kqs[sc][64:128, :]
                )
            for sc in range(NCH):
                for r in range(2):
                    cols = slice(r * SH + sc * CW, r * SH + (sc + 1) * CW)
                    nc.gpsimd.dma_start(
                        kt_sb[0:64, cols],
                        cc_out[sc][r, 0 : 64 * CW].rearrange("(p f) -> p f", p=64),
                    )
                    nc.gpsimd.dma_start(
                        vT_sb[:, cols],
                        cc_out[sc][r, 64 * CW : WKV].rearrange("(p f) -> p f", p=64),
                    )
        wstage_ctx.close()

        # kj visit order: tiles in AG-chunk completion order
        kpc = CW // 128  # kj tiles per AG chunk per half
        kj_order = []
        for c in range(NCH):
            kj_order += list(range(c * kpc, (c + 1) * kpc))
            kj_order += list(range(16 + c * kpc, 16 + (c + 1) * kpc))

        # ---- Phase B/C: attention + output projection -------------------
        # first chunk at QC=1024 (best exp amortization), second chunk as
        # two 512-wide subchunks so the exposed end-of-kernel tail is short
        with ExitStack() as pb:
            sps = pb.enter_context(tc.tile_pool(name="sps", bufs=2, space="PSUM"))
            cps = pb.enter_context(tc.tile_pool(name="cps", bufs=1, space="PSUM"))
            expp = pb.enter_context(tc.tile_pool(name="expp", bufs=8))
            ctxp = pb.enter_context(tc.tile_pool(name="ctxp", bufs=2))
            rsp = pb.enter_context(tc.tile_pool(name="rsp", bufs=2))
            outp = pb.enter_context(tc.tile_pool(name="outp", bufs=4))

            first_visit = [True]

            def emit_attn(q0, w):
                """attention for queries [q0, q0+w); returns ctx PSUM tile"""
                ctx = cps.tile([128, QC], F32, name="ctx", tag="ctx")
                for i, kj in enumerate(kj_order):
                    sc_ps = sps.tile([128, QC], F32, name="sc_ps", tag="sc")
                    lhs_k = kt_sb[:, kj * 128 : (kj + 1) * 128]
                    for n in range(w // 512):
                        nc.tensor.matmul(
                            sc_ps[:, n * 512 : (n + 1) * 512],
                            lhs_k,
                            q2_sb[:, q0 + n * 512 : q0 + (n + 1) * 512],
                        )
                    ex = expp.tile([128, QC], BF16, name="ex", tag="ex")
                    nc.scalar.activation(
                        ex[:, 0:w], sc_ps[:, 0:w], AF.Exp, scale=SCALE
                    )
                    if first_visit[0]:  # v natural tile, first use
                        p = mps.tile([128, 64], F32, tag="mp", name="vtp")
                        nc.tensor.matmul(
                            p[:],
                            vT_sb[:, kj * 128 : (kj + 1) * 128],
                            ident[0:64, 0:64],
                        )
                        nc.vector.tensor_copy(
                            v_aug[:, kj * 128 : kj * 128 + 64], p[:]
                        )
                    lhs_v = v_aug[:, kj * 128 : (kj + 1) * 128]
                    for n in range(w // 512):
                        nc.tensor.matmul(
                            ctx[:, n * 512 : (n + 1) * 512],
                            lhs_v,
                            ex[:, n * 512 : (n + 1) * 512],
                            start=(i == 0), stop=(i == ST - 1),
                            skip_group_check=True,
                        )
                first_visit[0] = False
                return ctx

            def emit_phc(ctx, q0, w):
                """output projection for queries [q0, q0+w)"""
                ctx_sb = ctxp.tile([65, QC], F32, tag="ctx32", name="ctx_sb")
                nc.vector.tensor_copy(ctx_sb[:, 0:w], ctx[0:65, 0:w])
                ctx_b16 = ctxp.tile([128, QC], BF16, tag="ctx16", name="ctx_b16")
                nc.gpsimd.memset(ctx_b16[64:128, 0:w], 0.0)
                nc.vector.tensor_copy(ctx_b16[0:65, 0:w], ctx_sb[:, 0:w])
                rs_row = rsp.tile([1, QC], BF16, tag="rsrow", name="rs_row")
                nc.sync.dma_start(rs_row[:, 0:w], ctx_b16[64:65, 0:w])

                rs_ps = mps.tile([128, QC // 128], F32, tag="mp", name="rsps")
                for c in range(w // 128):
                    nc.tensor.matmul(
                        rs_ps[:, c : c + 1],
                        rs_row[0:1, c * 128 : (c + 1) * 128],
                        ones11[:],
                    )
                recip = rsp.tile([128, QC // 128], F32, tag="recip", name="recip")
                nc.vector.reciprocal(recip[:, 0 : w // 128], rs_ps[:, 0 : w // 128])

                for c in range(w // 128):
                    out_sb = outp.tile([128, E], F32, name="out_sb")
                    for n in range(2):
                        op = mps.tile([128, 512], F32, tag="mp", name="opps")
                        nc.tensor.matmul(
                            op[:],
                            ctx_b16[:, c * 128 : (c + 1) * 128],
                            wo_sb[:, n * 512 : (n + 1) * 512],
                        )
                        nc.vector.tensor_scalar_mul(
                            out_sb[:, n * 512 : (n + 1) * 512],
                            op[:],
                            recip[:, c : c + 1],
                        )
                    (nc.sync if c % 2 == 0 else nc.scalar).dma_start(
                        out_ext[q0 + c * 128 : q0 + (c + 1) * 128, :], out_sb[:]
                    )

            ctx0 = emit_attn(0, 1024)
            emit_phc(ctx0, 0, 1024)
            ctx1 = emit_attn(1024, 512)
            emit_phc(ctx1, 1024, 512)
            ctx2 = emit_attn(1536, 512)
            emit_phc(ctx2, 1536, 512)


_NC = None


def _get_nc():
    global _NC
    if _NC is None:
        nc = bacc.Bacc("TRN2", target_bir_lowering=False, debug=False,
                       num_devices=N_CORES)
        x_ext = nc.dram_tensor("x", [SH, E], F32, kind="ExternalInput").ap()
        wq_ext = nc.dram_tensor("w_qkv", [E, D3], F32, kind="ExternalInput").ap()
        bq_ext = nc.dram_tensor("b_qkv", [D3], F32, kind="ExternalInput").ap()
        wo_ext = nc.dram_tensor("w_out", [H, E], F32, kind="ExternalInput").ap()
        bo_ext = nc.dram_tensor("b_out", [E], F32, kind="ExternalInput").ap()
        out_ext = nc.dram_tensor("out", [SH, E], F32, kind="ExternalOutput").ap()
        with tile.TileContext(nc) as tc:
            _emit(nc, tc, x_ext, wq_ext, bq_ext, wo_ext, bo_ext, out_ext)
        nc.compile()
        _NC = nc
    return _NC


last_results = None
last_tmpdir = None


def kernel(x, W_qkv, b_qkv, W_out, b_out):
    nc = _get_nc()
    x = np.ascontiguousarray(x, dtype=np.float32)
    shared = {
        "w_qkv": np.ascontiguousarray(W_qkv, dtype=np.float32),
        "b_qkv": np.ascontiguousarray(b_qkv, dtype=np.float32),
        "w_out": np.ascontiguousarray(W_out, dtype=np.float32),
        "b_out": np.ascontiguousarray(b_out, dtype=np.float32),
    }
    in_maps = []
    for c in range(N_CORES):
        b, h = divmod(c, 2)
        xp = np.ascontiguousarray(x[b, h * SH : (h + 1) * SH])
        in_maps.append({"x": xp, **shared})

    import os
    import tempfile
    import time

    tmpdir = os.environ.get("ATTN_TRACE_DIR") or tempfile.mkdtemp(prefix="attn_trace_")
    res = None
    for attempt in range(3):
        try:
            res = run_bass_kernel_spmd(
                nc, in_maps, core_ids=list(range(N_CORES)), tmpdir=tmpdir
            )
            break
        except Exception:
            # transient NRT_EXEC_UNIT_UNRECOVERABLE has been observed on a
            # first attempt; a clean retry recovers
            if attempt == 2:
                raise
            time.sleep(2.0)
    global last_results, last_tmpdir
    last_results = res
    last_tmpdir = tmpdir

    out = np.empty((B, S, E), dtype=np.float32)
    for c in range(N_CORES):
        b, h = divmod(c, 2)
        out[b, h * SH : (h + 1) * SH] = res.results[c]["out"]
    return out
